# revision 6
# baseline (speedup 1.0000x reference)
"""Trainium2 Bass kernel for nn_MoELayer (top-2 MoE, E=8 experts).

Strategy (expert-parallel across 8 NeuronCores):
  - Host computes the (tiny) gate matmul + top-2 + softmax, and dispatches
    each token to its two experts' cores ("all-to-all" done host-side as the
    sharding step). One expert per core.
  - Each core runs a Bass kernel computing, for its expert e and its routed
    tokens:   out = (silu(tok @ W1[e]) @ W2[e]) * gate_weight
    with bf16 matmul inputs and fp32 PSUM accumulation. Weights stay
    resident in SBUF; only the top-2-selected tokens are computed
    (4x fewer FLOPs than the dense reference).
  - Host scatter-adds the two weighted expert outputs per token.

Layouts (chosen so no on-device transposes are needed):
  stage 1:  actT[f, c] = silu( sum_d W1[d, f] * tokT[d, c] )
            matmul(lhsT=W1[dk, fj-tile], rhs=tokT[dk, c-chunk]) -> PSUM [f, c]
  stage 2:  out[c, d] = sum_f actT[f, c] * W2[f, d]
            matmul(lhsT=actT[fk, c-tile], rhs=W2[fk, d-chunk]) -> PSUM [c, d]

Perf notes (from perfetto trace analysis):
  - Weights are stored f-major ([P, KF, KD, 128]) so the DMA stream can be
    issued in f-block granularity in exactly the order stage 1/2 consume
    them; the first matmul only waits for chunk-0 tokens + one 256KB
    f-block instead of a 2MB quarter.
  - A short burst of warmup matmuls on a memset scratch tile spins the PE
    out of its low p-state while the first real DMAs land.
  - C (token capacity per core) is the exact max routed-token count.
"""

import math
import sys

sys.path.insert(0, "/opt/trn_rl_repo")

import ml_dtypes
import numpy as np

B, T, D, F, E = 2, 2048, 1024, 4096, 8
N = B * T
P = 128
KD = D // P  # 8
KF = F // P  # 32

bf16 = ml_dtypes.bfloat16

_nc_cache: dict[int, object] = {}
LAST_RESULTS = None  # BassKernelResults from the most recent run (for test.py)
TRACE = False


def _chunk_sizes(C: int) -> list[int]:
    """Split C into near-equal chunks of <=512 (stage-1 matmul free dim /
    PSUM bank limit), smallest first so the critical first token transfer
    is as small as possible."""
    n = math.ceil(C / 512)
    base = math.ceil(C / (n * P)) * P
    sizes = []
    rem = C
    while rem > 0:
        s = min(base, rem)
        sizes.append(s)
        rem -= s
    return sorted(sizes)


def _build(C: int):
    import concourse.mybir as mybir
    import concourse.tile as tile
    from concourse import bacc

    dt = mybir.dt

    nc = bacc.Bacc(None, target_bir_lowering=False)

    chunks = _chunk_sizes(C)

    # one token tensor per chunk -> fully contiguous per-partition DMA
    # packets (KD*cn*2 bytes) instead of 768B strided slices
    tokts = [
        nc.dram_tensor(f"tokt{i}", [P, KD, cn], dt.bfloat16, kind="ExternalInput")
        for i, cn in enumerate(chunks)
    ]
    # f-major weight layouts: per-f-block slices are contiguous per
    # partition, so the DMA stream can follow consumption order exactly.
    w1 = nc.dram_tensor("w1", [P, KF, KD, P], dt.bfloat16, kind="ExternalInput")
    w2 = nc.dram_tensor("w2", [P, KF, KD, P], dt.bfloat16, kind="ExternalInput")
    # output is transposed: [D, C] with D on partitions; the gate-weight
    # scale + transpose happen on the host during scatter-add
    out = nc.dram_tensor("out", [D, C], dt.float32, kind="ExternalOutput")

    with tile.TileContext(nc) as tc:
        with (
            tc.tile_pool(name="const", bufs=1) as cpool,
            tc.tile_pool(name="act", bufs=1) as apool,
            tc.tile_pool(name="warm", bufs=1) as wpool,
            tc.tile_pool(name="ps1", bufs=2, space="PSUM") as ps1pool,
            tc.tile_pool(name="ps2", bufs=2, space="PSUM") as ps2pool,
            tc.tile_pool(name="psw", bufs=1, space="PSUM") as pswpool,
            tc.tile_pool(name="ob", bufs=4) as opool,
        ):
            w1_sb = cpool.tile([P, KF, KD, P], dt.bfloat16, tag="w1")
            w2_sb = cpool.tile([P, KF, KD, P], dt.bfloat16, tag="w2")
            tok_sbs = [
                cpool.tile(
                    [P, KD, cn], dt.bfloat16, tag=f"tok{i}", name=f"tok_sb{i}"
                )
                for i, cn in enumerate(chunks)
            ]

            # ---- PE warmup: spin the tensor engine out of its low p-state
            # while the first input DMAs are still in flight (the PE only
            # reaches max clock after ~3us of continuous execution).
            warm_sb = wpool.tile([P, 512], dt.bfloat16, tag="warm")
            nc.gpsimd.memset(warm_sb[:], 0.0)
            ps_w = pswpool.tile([P, 512], dt.float32, tag="psw")
            NWARM = 7
            for wi in range(NWARM):
                nc.tensor.matmul(
                    ps_w[:],
                    warm_sb[:, 0:P],
                    warm_sb[:],
                    start=(wi == 0),
                    stop=(wi == NWARM - 1),
                )

            # Input loads split across both HW DGE queues: sync carries
            # chunk-0 tokens + the W1 stream (fine-grained at the head so
            # the first matmul waits on ~256KB) + remaining tokens; the
            # scalar engine's DGE carries W2 in parallel so it lands before
            # stage 2 of chunk 0 instead of queuing behind all of W1.
            nc.sync.dma_start(tok_sbs[0][:], tokts[0][:])
            for q in range(4):
                nc.scalar.dma_start(
                    w2_sb[:, q * (KF // 4) : (q + 1) * (KF // 4)],
                    w2[:, q * (KF // 4) : (q + 1) * (KF // 4)],
                )
            w1_groups = [(0, 1), (1, 2), (2, 4), (4, 8), (8, 16), (16, 24), (24, 32)]
            for a, b in w1_groups:
                nc.sync.dma_start(w1_sb[:, a:b], w1[:, a:b])
            for i in range(1, len(chunks)):
                nc.sync.dma_start(tok_sbs[i][:], tokts[i][:])

            c0 = 0
            for ci, cn in enumerate(chunks):
                tok_sb = tok_sbs[ci]
                act_sb = apool.tile([P, KF, cn], dt.bfloat16, tag="act")
                # ---- stage 1: actT = silu(W1^T @ tokT) ----
                for fj in range(KF):
                    ps1 = ps1pool.tile([P, cn], dt.float32, tag="ps1")
                    for dk in range(KD):
                        nc.tensor.matmul(
                            ps1[:],
                            w1_sb[:, fj, dk, :],
                            tok_sb[:, dk, :],
                            start=(dk == 0),
                            stop=(dk == KD - 1),
                        )
                    nc.scalar.activation(
                        act_sb[:, fj, :],
                        ps1[:],
                        mybir.ActivationFunctionType.Silu,
                    )
                # ---- stage 2: outT = W2^T @ actT  (D on partitions,
                # tokens on the free dim -> no padded token tiles) ----
                last_chunk = ci == len(chunks) - 1
                for dm in range(D // P):
                    ps2 = ps2pool.tile([P, cn], dt.float32, tag="ps2")
                    for fk in range(KF):
                        nc.tensor.matmul(
                            ps2[:],
                            w2_sb[:, fk, dm, :],
                            act_sb[:, fk, :],
                            start=(fk == 0),
                            stop=(fk == KF - 1),
                        )
                    # split the very last output tile so the tail DMA after
                    # the final matmul is half-size
                    if last_chunk and dm == D // P - 1 and cn > 128:
                        h = cn // 2
                        for s, e_ in ((0, h), (h, cn)):
                            ob = opool.tile([P, e_ - s], dt.float32, tag="ob")
                            nc.vector.tensor_copy(ob[:], ps2[:, s:e_])
                            nc.sync.dma_start(
                                out[dm * P : (dm + 1) * P, c0 + s : c0 + e_],
                                ob[:],
                            )
                    else:
                        ob = opool.tile([P, cn], dt.float32, tag="ob")
                        nc.vector.tensor_copy(ob[:], ps2[:])
                        nc.sync.dma_start(
                            out[dm * P : (dm + 1) * P, c0 : c0 + cn],
                            ob[:],
                        )
                c0 += cn

    nc.compile()
    return nc


def _get_nc(C: int):
    if C not in _nc_cache:
        _nc_cache[C] = _build(C)
    return _nc_cache[C]


def kernel(**inputs) -> np.ndarray:
    global LAST_RESULTS
    x = np.asarray(inputs["x"], dtype=np.float32)
    Wg = np.asarray(inputs["Wg"], dtype=np.float32)
    W1 = np.asarray(inputs["W1"], dtype=np.float32)
    W2 = np.asarray(inputs["W2"], dtype=np.float32)

    h = np.ascontiguousarray(x.reshape(N, D))

    # ---- host gate: top-2 + softmax (0.05% of total FLOPs) ----
    logits = h @ Wg.T  # [N, E] f32
    idx2 = np.argpartition(-logits, 1, axis=1)[:, :2]
    lsel = np.take_along_axis(logits, idx2, axis=1)
    first = lsel[:, 0] >= lsel[:, 1]
    i0 = np.where(first, idx2[:, 0], idx2[:, 1])
    i1 = np.where(first, idx2[:, 1], idx2[:, 0])
    l0 = np.where(first, lsel[:, 0], lsel[:, 1])
    l1 = np.where(first, lsel[:, 1], lsel[:, 0])
    e1 = np.exp((l1 - l0).astype(np.float32))
    w0 = (1.0 / (1.0 + e1)).astype(np.float32)
    w1g = (e1 / (1.0 + e1)).astype(np.float32)

    token_ids = np.concatenate([np.arange(N), np.arange(N)])
    expert_ids = np.concatenate([i0, i1])
    gate_w = np.concatenate([w0, w1g])

    counts = np.bincount(expert_ids, minlength=E)
    C = int(counts.max())

    hb = h.astype(bf16)
    W1b = W1.astype(bf16)
    W2b = W2.astype(bf16)

    in_maps = []
    ids_per_expert = []
    gw_per_expert = []
    for e in range(E):
        sel = np.flatnonzero(expert_ids == e)
        ids_e = token_ids[sel]
        n_e = len(ids_e)
        ids_per_expert.append(ids_e)
        gw_per_expert.append(gate_w[sel])

        tokT = np.zeros((P, KD, C), dtype=bf16)
        # tokens [n,D] -> [D,n] -> [KD,P,n] -> [P,KD,n]
        tokT[:, :, :n_e] = (
            hb[ids_e].T.reshape(KD, P, n_e).transpose(1, 0, 2)
        )
        m = {
            # f-major: w1t[p, fj, dk, fc] = W1[dk*P+p, fj*P+fc]
            "w1": np.ascontiguousarray(
                W1b[e].reshape(KD, P, KF, P).transpose(1, 2, 0, 3)
            ),
            # f-major: w2t[p, fk, dm, dc] = W2[fk*P+p, dm*P+dc]
            "w2": np.ascontiguousarray(
                W2b[e].reshape(KF, P, KD, P).transpose(1, 0, 2, 3)
            ),
        }
        c0 = 0
        for i, cn in enumerate(_chunk_sizes(C)):
            m[f"tokt{i}"] = np.ascontiguousarray(tokT[:, :, c0 : c0 + cn])
            c0 += cn
        in_maps.append(m)

    nc = _get_nc(C)
    from concourse.bass_utils import run_bass_kernel_spmd

    LAST_RESULTS = run_bass_kernel_spmd(
        nc, in_maps, core_ids=list(range(E)), trace=TRACE
    )

    y = np.zeros((N, D), dtype=np.float32)
    for e in range(E):
        o = np.asarray(LAST_RESULTS.results[e]["out"], dtype=np.float32)  # [D, C]
        ids_e = ids_per_expert[e]
        n_e = len(ids_e)
        y[ids_e] += gw_per_expert[e][:, None] * o[:, :n_e].T
    return y.reshape(B, T, D)


# revision 9
# speedup vs baseline: 1.1030x; 1.1030x over previous
"""Trainium2 Bass kernel for nn_MoELayer (top-2 MoE, E=8 experts).

Strategy (expert-parallel across 8 NeuronCores):
  - Host computes the (tiny) gate matmul + top-2 + softmax, and dispatches
    each token to its two experts' cores ("all-to-all" done host-side as the
    sharding step). One expert per core.
  - Each core runs a Bass kernel computing, for its expert e and its routed
    tokens:   out = (silu(tok @ W1[e]) @ W2[e]) * gate_weight
    with bf16 matmul inputs and fp32 PSUM accumulation. Weights stay
    resident in SBUF; only the top-2-selected tokens are computed
    (4x fewer FLOPs than the dense reference).
  - Host scatter-adds the two weighted expert outputs per token.

Layouts (chosen so no on-device transposes are needed):
  stage 1:  actT[f, c] = silu( sum_d W1[d, f] * tokT[d, c] )
            matmul(lhsT=W1[dk, fj-tile], rhs=tokT[dk, c-chunk]) -> PSUM [f, c]
  stage 2:  out[c, d] = sum_f actT[f, c] * W2[f, d]
            matmul(lhsT=actT[fk, c-tile], rhs=W2[fk, d-chunk]) -> PSUM [c, d]

Perf notes (from perfetto trace analysis):
  - Weights are stored f-major ([P, KF, KD, 128]) so the DMA stream can be
    issued in f-block granularity in exactly the order stage 1/2 consume
    them; the first matmul only waits for chunk-0 tokens + one 256KB
    f-block instead of a 2MB quarter.
  - A short burst of warmup matmuls on a memset scratch tile spins the PE
    out of its low p-state while the first real DMAs land.
  - C (token capacity per core) is the exact max routed-token count.
"""

import math
import sys

sys.path.insert(0, "/opt/trn_rl_repo")

import ml_dtypes
import numpy as np

B, T, D, F, E = 2, 2048, 1024, 4096, 8
N = B * T
P = 128
KD = D // P  # 8
KF = F // P  # 32

bf16 = ml_dtypes.bfloat16

_nc_cache: dict[int, object] = {}
LAST_RESULTS = None  # BassKernelResults from the most recent run (for test.py)
TRACE = False


def _chunk_sizes(C: int) -> list[int]:
    """Split C into near-equal chunks of <=512 (stage-1 matmul free dim /
    PSUM bank limit), smallest first so the critical first token transfer
    is as small as possible."""
    n = math.ceil(C / 512)
    base = math.ceil(C / (n * P)) * P
    sizes = []
    rem = C
    while rem > 0:
        s = min(base, rem)
        sizes.append(s)
        rem -= s
    # descending: a big first chunk keeps early weight-DMA deadlines loose
    # (stage-2 starts later), and a small last chunk shortens the tail
    return sorted(sizes, reverse=True)


def _build(C: int):
    import concourse.mybir as mybir
    import concourse.tile as tile
    from concourse import bacc

    dt = mybir.dt

    nc = bacc.Bacc(None, target_bir_lowering=False)

    chunks = _chunk_sizes(C)

    # one token tensor per chunk -> fully contiguous per-partition DMA
    # packets (KD*cn*2 bytes) instead of 768B strided slices
    tokts = [
        nc.dram_tensor(f"tokt{i}", [P, KD, cn], dt.bfloat16, kind="ExternalInput")
        for i, cn in enumerate(chunks)
    ]
    # f-major weight layouts: per-f-block slices are contiguous per
    # partition, so the DMA stream can follow consumption order exactly.
    w1 = nc.dram_tensor("w1", [P, KF, KD, P], dt.bfloat16, kind="ExternalInput")
    w2 = nc.dram_tensor("w2", [P, KF, KD, P], dt.bfloat16, kind="ExternalInput")
    # output is transposed: [D, C] with D on partitions; the gate-weight
    # scale + transpose happen on the host during scatter-add
    out = nc.dram_tensor("out", [D, C], dt.float32, kind="ExternalOutput")

    with tile.TileContext(nc) as tc:
        with (
            tc.tile_pool(name="const", bufs=1) as cpool,
            tc.tile_pool(name="act", bufs=1) as apool,
            tc.tile_pool(name="warm", bufs=1) as wpool,
            tc.tile_pool(name="ps1", bufs=3, space="PSUM") as ps1pool,
            tc.tile_pool(name="ps2", bufs=2, space="PSUM") as ps2pool,
            tc.tile_pool(name="psw", bufs=1, space="PSUM") as pswpool,
            tc.tile_pool(name="ob", bufs=4) as opool,
        ):
            w1_sb = cpool.tile([P, KF, KD, P], dt.bfloat16, tag="w1")
            w2_sb = cpool.tile([P, KF, KD, P], dt.bfloat16, tag="w2")
            tok_sbs = [
                cpool.tile(
                    [P, KD, cn], dt.bfloat16, tag=f"tok{i}", name=f"tok_sb{i}"
                )
                for i, cn in enumerate(chunks)
            ]

            # ---- PE warmup: spin the tensor engine out of its low p-state
            # while the first input DMAs are still in flight (the PE only
            # reaches max clock after ~3us of continuous execution).
            warm_sb = wpool.tile([P, 512], dt.bfloat16, tag="warm")
            nc.gpsimd.memset(warm_sb[:], 0.0)
            ps_w = pswpool.tile([P, 512], dt.float32, tag="psw")
            NWARM = 7
            for wi in range(NWARM):
                nc.tensor.matmul(
                    ps_w[:],
                    warm_sb[:, 0:P],
                    warm_sb[:],
                    start=(wi == 0),
                    stop=(wi == NWARM - 1),
                )

            # Input loads all on the sync engine's HW DGE (the scalar
            # engine's DGE queue stalls the activation stream behind it),
            # earliest-deadline order. tok0 is split by dk and W1 is
            # fine-grained at the head so the first matmul gates on ~350KB.
            nc.sync.dma_start(tok_sbs[0][:, 0:1], tokts[0][:, 0:1])
            nc.sync.dma_start(w1_sb[:, 0:1], w1[:, 0:1])
            nc.sync.dma_start(tok_sbs[0][:, 1:4], tokts[0][:, 1:4])
            nc.sync.dma_start(tok_sbs[0][:, 4:KD], tokts[0][:, 4:KD])
            w1_groups = [(1, 2), (2, 4), (4, 8), (8, 16), (16, 24), (24, 32)]
            for a, b in w1_groups:
                nc.sync.dma_start(w1_sb[:, a:b], w1[:, a:b])
            for q in range(4):
                nc.sync.dma_start(
                    w2_sb[:, q * (KF // 4) : (q + 1) * (KF // 4)],
                    w2[:, q * (KF // 4) : (q + 1) * (KF // 4)],
                )
            for i in range(1, len(chunks)):
                nc.sync.dma_start(tok_sbs[i][:], tokts[i][:])

            c0 = 0
            for ci, cn in enumerate(chunks):
                tok_sb = tok_sbs[ci]
                act_sb = apool.tile([P, KF, cn], dt.bfloat16, tag="act")
                # ---- stage 1: actT = silu(W1^T @ tokT) ----
                for fj in range(KF):
                    ps1 = ps1pool.tile([P, cn], dt.float32, tag="ps1")
                    for dk in range(KD):
                        nc.tensor.matmul(
                            ps1[:],
                            w1_sb[:, fj, dk, :],
                            tok_sb[:, dk, :],
                            start=(dk == 0),
                            stop=(dk == KD - 1),
                        )
                    nc.scalar.activation(
                        act_sb[:, fj, :],
                        ps1[:],
                        mybir.ActivationFunctionType.Silu,
                    )
                # ---- stage 2: outT = W2^T @ actT  (D on partitions,
                # tokens on the free dim -> no padded token tiles) ----
                last_chunk = ci == len(chunks) - 1
                for dm in range(D // P):
                    ps2 = ps2pool.tile([P, cn], dt.float32, tag="ps2")
                    for fk in range(KF):
                        nc.tensor.matmul(
                            ps2[:],
                            w2_sb[:, fk, dm, :],
                            act_sb[:, fk, :],
                            start=(fk == 0),
                            stop=(fk == KF - 1),
                        )
                    # split the very last output tile so the tail DMA after
                    # the final matmul is half-size
                    if last_chunk and dm == D // P - 1 and cn > 128:
                        h = cn // 2
                        for s, e_ in ((0, h), (h, cn)):
                            ob = opool.tile([P, e_ - s], dt.float32, tag="ob")
                            nc.vector.tensor_copy(ob[:], ps2[:, s:e_])
                            nc.sync.dma_start(
                                out[dm * P : (dm + 1) * P, c0 + s : c0 + e_],
                                ob[:],
                            )
                    else:
                        ob = opool.tile([P, cn], dt.float32, tag="ob")
                        nc.vector.tensor_copy(ob[:], ps2[:])
                        nc.sync.dma_start(
                            out[dm * P : (dm + 1) * P, c0 : c0 + cn],
                            ob[:],
                        )
                c0 += cn

    nc.compile()
    return nc


def _get_nc(C: int):
    if C not in _nc_cache:
        _nc_cache[C] = _build(C)
    return _nc_cache[C]


def kernel(**inputs) -> np.ndarray:
    global LAST_RESULTS
    x = np.asarray(inputs["x"], dtype=np.float32)
    Wg = np.asarray(inputs["Wg"], dtype=np.float32)
    W1 = np.asarray(inputs["W1"], dtype=np.float32)
    W2 = np.asarray(inputs["W2"], dtype=np.float32)

    h = np.ascontiguousarray(x.reshape(N, D))

    # ---- host gate: top-2 + softmax (0.05% of total FLOPs) ----
    logits = h @ Wg.T  # [N, E] f32
    idx2 = np.argpartition(-logits, 1, axis=1)[:, :2]
    lsel = np.take_along_axis(logits, idx2, axis=1)
    first = lsel[:, 0] >= lsel[:, 1]
    i0 = np.where(first, idx2[:, 0], idx2[:, 1])
    i1 = np.where(first, idx2[:, 1], idx2[:, 0])
    l0 = np.where(first, lsel[:, 0], lsel[:, 1])
    l1 = np.where(first, lsel[:, 1], lsel[:, 0])
    e1 = np.exp((l1 - l0).astype(np.float32))
    w0 = (1.0 / (1.0 + e1)).astype(np.float32)
    w1g = (e1 / (1.0 + e1)).astype(np.float32)

    token_ids = np.concatenate([np.arange(N), np.arange(N)])
    expert_ids = np.concatenate([i0, i1])
    gate_w = np.concatenate([w0, w1g])

    counts = np.bincount(expert_ids, minlength=E)
    C = int(counts.max())

    hb = h.astype(bf16)
    W1b = W1.astype(bf16)
    W2b = W2.astype(bf16)

    in_maps = []
    ids_per_expert = []
    gw_per_expert = []
    for e in range(E):
        sel = np.flatnonzero(expert_ids == e)
        ids_e = token_ids[sel]
        n_e = len(ids_e)
        ids_per_expert.append(ids_e)
        gw_per_expert.append(gate_w[sel])

        tokT = np.zeros((P, KD, C), dtype=bf16)
        # tokens [n,D] -> [D,n] -> [KD,P,n] -> [P,KD,n]
        tokT[:, :, :n_e] = (
            hb[ids_e].T.reshape(KD, P, n_e).transpose(1, 0, 2)
        )
        m = {
            # f-major: w1t[p, fj, dk, fc] = W1[dk*P+p, fj*P+fc]
            "w1": np.ascontiguousarray(
                W1b[e].reshape(KD, P, KF, P).transpose(1, 2, 0, 3)
            ),
            # f-major: w2t[p, fk, dm, dc] = W2[fk*P+p, dm*P+dc]
            "w2": np.ascontiguousarray(
                W2b[e].reshape(KF, P, KD, P).transpose(1, 0, 2, 3)
            ),
        }
        c0 = 0
        for i, cn in enumerate(_chunk_sizes(C)):
            m[f"tokt{i}"] = np.ascontiguousarray(tokT[:, :, c0 : c0 + cn])
            c0 += cn
        in_maps.append(m)

    nc = _get_nc(C)
    from concourse.bass_utils import run_bass_kernel_spmd

    LAST_RESULTS = run_bass_kernel_spmd(
        nc, in_maps, core_ids=list(range(E)), trace=TRACE
    )

    y = np.zeros((N, D), dtype=np.float32)
    for e in range(E):
        o = np.asarray(LAST_RESULTS.results[e]["out"], dtype=np.float32)  # [D, C]
        ids_e = ids_per_expert[e]
        n_e = len(ids_e)
        y[ids_e] += gw_per_expert[e][:, None] * o[:, :n_e].T
    return y.reshape(B, T, D)


# revision 10
# speedup vs baseline: 1.1267x; 1.0216x over previous
"""Trainium2 Bass kernel for nn_MoELayer (top-2 MoE, E=8 experts).

Strategy (expert-parallel across 8 NeuronCores):
  - Host computes the (tiny) gate matmul + top-2 + softmax, and dispatches
    each token to its two experts' cores ("all-to-all" done host-side as the
    sharding step). One expert per core.
  - Each core runs a Bass kernel computing, for its expert e and its routed
    tokens:   out = (silu(tok @ W1[e]) @ W2[e]) * gate_weight
    with bf16 matmul inputs and fp32 PSUM accumulation. Weights stay
    resident in SBUF; only the top-2-selected tokens are computed
    (4x fewer FLOPs than the dense reference).
  - Host scatter-adds the two weighted expert outputs per token.

Layouts (chosen so no on-device transposes are needed):
  stage 1:  actT[f, c] = silu( sum_d W1[d, f] * tokT[d, c] )
            matmul(lhsT=W1[dk, fj-tile], rhs=tokT[dk, c-chunk]) -> PSUM [f, c]
  stage 2:  out[c, d] = sum_f actT[f, c] * W2[f, d]
            matmul(lhsT=actT[fk, c-tile], rhs=W2[fk, d-chunk]) -> PSUM [c, d]

Perf notes (from perfetto trace analysis):
  - Weights are stored f-major ([P, KF, KD, 128]) so the DMA stream can be
    issued in f-block granularity in exactly the order stage 1/2 consume
    them; the first matmul only waits for chunk-0 tokens + one 256KB
    f-block instead of a 2MB quarter.
  - A short burst of warmup matmuls on a memset scratch tile spins the PE
    out of its low p-state while the first real DMAs land.
  - C (token capacity per core) is the exact max routed-token count.
"""

import math
import sys

sys.path.insert(0, "/opt/trn_rl_repo")

import ml_dtypes
import numpy as np

B, T, D, F, E = 2, 2048, 1024, 4096, 8
N = B * T
P = 128
KD = D // P  # 8
KF = F // P  # 32

bf16 = ml_dtypes.bfloat16

_nc_cache: dict[int, object] = {}
LAST_RESULTS = None  # BassKernelResults from the most recent run (for test.py)
TRACE = False


def _chunk_sizes(C: int) -> list[int]:
    """Split C into near-equal chunks of <=512 (stage-1 matmul free dim /
    PSUM bank limit), smallest first so the critical first token transfer
    is as small as possible."""
    n = math.ceil(C / 512)
    base = math.ceil(C / (n * P)) * P
    sizes = []
    rem = C
    while rem > 0:
        s = min(base, rem)
        sizes.append(s)
        rem -= s
    # descending: a big first chunk keeps early weight-DMA deadlines loose
    # (stage-2 starts later), and a small last chunk shortens the tail
    return sorted(sizes, reverse=True)


def _build(C: int):
    import concourse.mybir as mybir
    import concourse.tile as tile
    from concourse import bacc

    dt = mybir.dt

    nc = bacc.Bacc(None, target_bir_lowering=False)

    chunks = _chunk_sizes(C)

    # one token tensor per chunk -> fully contiguous per-partition DMA
    # packets (KD*cn*2 bytes) instead of 768B strided slices
    tokts = [
        nc.dram_tensor(f"tokt{i}", [P, KD, cn], dt.bfloat16, kind="ExternalInput")
        for i, cn in enumerate(chunks)
    ]
    # f-major weight layouts: per-f-block slices are contiguous per
    # partition, so the DMA stream can follow consumption order exactly.
    w1 = nc.dram_tensor("w1", [P, KF, KD, P], dt.bfloat16, kind="ExternalInput")
    w2 = nc.dram_tensor("w2", [P, KF, KD, P], dt.bfloat16, kind="ExternalInput")
    # output is transposed: [D, C] with D on partitions; the gate-weight
    # scale + transpose happen on the host during scatter-add
    out = nc.dram_tensor("out", [D, C], dt.float32, kind="ExternalOutput")

    with tile.TileContext(nc) as tc:
        with (
            tc.tile_pool(name="const", bufs=1) as cpool,
            tc.tile_pool(name="act", bufs=1) as apool,
            tc.tile_pool(name="warm", bufs=1) as wpool,
            tc.tile_pool(name="ps1", bufs=3, space="PSUM") as ps1pool,
            tc.tile_pool(name="ps2", bufs=2, space="PSUM") as ps2pool,
            tc.tile_pool(name="psw", bufs=1, space="PSUM") as pswpool,
            tc.tile_pool(name="ob", bufs=4) as opool,
        ):
            w1_sb = cpool.tile([P, KF, KD, P], dt.bfloat16, tag="w1")
            w2_sb = cpool.tile([P, KF, KD, P], dt.bfloat16, tag="w2")
            tok_sbs = [
                cpool.tile(
                    [P, KD, cn], dt.bfloat16, tag=f"tok{i}", name=f"tok_sb{i}"
                )
                for i, cn in enumerate(chunks)
            ]

            # ---- PE warmup: spin the tensor engine out of its low p-state
            # while the first input DMAs are still in flight (the PE only
            # reaches max clock after ~3us of continuous execution).
            warm_sb = wpool.tile([P, 512], dt.bfloat16, tag="warm")
            nc.gpsimd.memset(warm_sb[:], 0.0)
            ps_w = pswpool.tile([P, 512], dt.float32, tag="psw")
            NWARM = 7
            for wi in range(NWARM):
                nc.tensor.matmul(
                    ps_w[:],
                    warm_sb[:, 0:P],
                    warm_sb[:],
                    start=(wi == 0),
                    stop=(wi == NWARM - 1),
                )

            # Input loads all on the sync engine's HW DGE (the scalar
            # engine's DGE queue stalls the activation stream behind it),
            # earliest-deadline order. tok0 is split by dk and W1 is
            # fine-grained at the head so the first matmul gates on ~350KB.
            nc.sync.dma_start(tok_sbs[0][:, 0:1], tokts[0][:, 0:1])
            nc.sync.dma_start(w1_sb[:, 0:1, 0:1], w1[:, 0:1, 0:1])
            nc.sync.dma_start(tok_sbs[0][:, 1:4], tokts[0][:, 1:4])
            nc.sync.dma_start(w1_sb[:, 0:1, 1:KD], w1[:, 0:1, 1:KD])
            nc.sync.dma_start(tok_sbs[0][:, 4:KD], tokts[0][:, 4:KD])
            w1_groups = [
                (1, 2), (2, 3), (3, 4), (4, 5), (5, 6), (6, 7), (7, 8),
                (8, 12), (12, 16), (16, 24), (24, 32),
            ]
            for a, b in w1_groups:
                nc.sync.dma_start(w1_sb[:, a:b], w1[:, a:b])
            for q in range(4):
                nc.sync.dma_start(
                    w2_sb[:, q * (KF // 4) : (q + 1) * (KF // 4)],
                    w2[:, q * (KF // 4) : (q + 1) * (KF // 4)],
                )
            for i in range(1, len(chunks)):
                nc.sync.dma_start(tok_sbs[i][:], tokts[i][:])

            c0 = 0
            for ci, cn in enumerate(chunks):
                tok_sb = tok_sbs[ci]
                act_sb = apool.tile([P, KF, cn], dt.bfloat16, tag="act")
                # ---- stage 1: actT = silu(W1^T @ tokT) ----
                for fj in range(KF):
                    ps1 = ps1pool.tile([P, cn], dt.float32, tag="ps1")
                    for dk in range(KD):
                        nc.tensor.matmul(
                            ps1[:],
                            w1_sb[:, fj, dk, :],
                            tok_sb[:, dk, :],
                            start=(dk == 0),
                            stop=(dk == KD - 1),
                        )
                    nc.scalar.activation(
                        act_sb[:, fj, :],
                        ps1[:],
                        mybir.ActivationFunctionType.Silu,
                    )
                # ---- stage 2: outT = W2^T @ actT  (D on partitions,
                # tokens on the free dim -> no padded token tiles) ----
                last_chunk = ci == len(chunks) - 1
                for dm in range(D // P):
                    ps2 = ps2pool.tile([P, cn], dt.float32, tag="ps2")
                    for fk in range(KF):
                        nc.tensor.matmul(
                            ps2[:],
                            w2_sb[:, fk, dm, :],
                            act_sb[:, fk, :],
                            start=(fk == 0),
                            stop=(fk == KF - 1),
                        )
                    # split the very last output tile so the tail DMA after
                    # the final matmul is half-size
                    if last_chunk and dm == D // P - 1 and cn > 128:
                        h = cn // 2
                        for s, e_ in ((0, h), (h, cn)):
                            ob = opool.tile([P, e_ - s], dt.float32, tag="ob")
                            nc.vector.tensor_copy(ob[:], ps2[:, s:e_])
                            nc.sync.dma_start(
                                out[dm * P : (dm + 1) * P, c0 + s : c0 + e_],
                                ob[:],
                            )
                    else:
                        ob = opool.tile([P, cn], dt.float32, tag="ob")
                        nc.vector.tensor_copy(ob[:], ps2[:])
                        nc.sync.dma_start(
                            out[dm * P : (dm + 1) * P, c0 : c0 + cn],
                            ob[:],
                        )
                c0 += cn

    nc.compile()
    return nc


def _get_nc(C: int):
    if C not in _nc_cache:
        _nc_cache[C] = _build(C)
    return _nc_cache[C]


def kernel(**inputs) -> np.ndarray:
    global LAST_RESULTS
    x = np.asarray(inputs["x"], dtype=np.float32)
    Wg = np.asarray(inputs["Wg"], dtype=np.float32)
    W1 = np.asarray(inputs["W1"], dtype=np.float32)
    W2 = np.asarray(inputs["W2"], dtype=np.float32)

    h = np.ascontiguousarray(x.reshape(N, D))

    # ---- host gate: top-2 + softmax (0.05% of total FLOPs) ----
    logits = h @ Wg.T  # [N, E] f32
    idx2 = np.argpartition(-logits, 1, axis=1)[:, :2]
    lsel = np.take_along_axis(logits, idx2, axis=1)
    first = lsel[:, 0] >= lsel[:, 1]
    i0 = np.where(first, idx2[:, 0], idx2[:, 1])
    i1 = np.where(first, idx2[:, 1], idx2[:, 0])
    l0 = np.where(first, lsel[:, 0], lsel[:, 1])
    l1 = np.where(first, lsel[:, 1], lsel[:, 0])
    e1 = np.exp((l1 - l0).astype(np.float32))
    w0 = (1.0 / (1.0 + e1)).astype(np.float32)
    w1g = (e1 / (1.0 + e1)).astype(np.float32)

    token_ids = np.concatenate([np.arange(N), np.arange(N)])
    expert_ids = np.concatenate([i0, i1])
    gate_w = np.concatenate([w0, w1g])

    counts = np.bincount(expert_ids, minlength=E)
    C = int(counts.max())

    hb = h.astype(bf16)
    W1b = W1.astype(bf16)
    W2b = W2.astype(bf16)

    in_maps = []
    ids_per_expert = []
    gw_per_expert = []
    for e in range(E):
        sel = np.flatnonzero(expert_ids == e)
        ids_e = token_ids[sel]
        n_e = len(ids_e)
        ids_per_expert.append(ids_e)
        gw_per_expert.append(gate_w[sel])

        tokT = np.zeros((P, KD, C), dtype=bf16)
        # tokens [n,D] -> [D,n] -> [KD,P,n] -> [P,KD,n]
        tokT[:, :, :n_e] = (
            hb[ids_e].T.reshape(KD, P, n_e).transpose(1, 0, 2)
        )
        m = {
            # f-major: w1t[p, fj, dk, fc] = W1[dk*P+p, fj*P+fc]
            "w1": np.ascontiguousarray(
                W1b[e].reshape(KD, P, KF, P).transpose(1, 2, 0, 3)
            ),
            # f-major: w2t[p, fk, dm, dc] = W2[fk*P+p, dm*P+dc]
            "w2": np.ascontiguousarray(
                W2b[e].reshape(KF, P, KD, P).transpose(1, 0, 2, 3)
            ),
        }
        c0 = 0
        for i, cn in enumerate(_chunk_sizes(C)):
            m[f"tokt{i}"] = np.ascontiguousarray(tokT[:, :, c0 : c0 + cn])
            c0 += cn
        in_maps.append(m)

    nc = _get_nc(C)
    from concourse.bass_utils import run_bass_kernel_spmd

    LAST_RESULTS = run_bass_kernel_spmd(
        nc, in_maps, core_ids=list(range(E)), trace=TRACE
    )

    y = np.zeros((N, D), dtype=np.float32)
    for e in range(E):
        o = np.asarray(LAST_RESULTS.results[e]["out"], dtype=np.float32)  # [D, C]
        ids_e = ids_per_expert[e]
        n_e = len(ids_e)
        y[ids_e] += gw_per_expert[e][:, None] * o[:, :n_e].T
    return y.reshape(B, T, D)


# revision 11
# speedup vs baseline: 1.1397x; 1.0115x over previous
"""Trainium2 Bass kernel for nn_MoELayer (top-2 MoE, E=8 experts).

Strategy v2 (F-sliced parallelism across 8 NeuronCores):
  - Host computes the gate matmul + top-2 + softmax and sorts the 8192
    (token, expert) pairs by expert.
  - Every core holds a 512-wide slice of the hidden dim F for ALL 8
    experts (W1[:, :, c*512:(c+1)*512] and W2[:, c*512:(c+1)*512, :],
    16MB bf16 total) and processes ALL 8192 pairs on its slice:
        partial_c = silu(tok @ W1s[e]) @ W2s[e]
    Host sums the 8 partials and scatter-adds with the gate weights.
  - This gives perfect load balance (exactly 8192 pairs per core, no
    padding) vs expert-parallel's max-expert-count padding, and spreads
    the weight DMA demand 8x thinner in time (2MB per expert segment
    instead of 16MB up front).

Layouts (no on-device transposes):
  stage 1:  actT[f, c] = silu( sum_d W1s[d, f] * tokT[d, c] )
  stage 2:  outT[d, c] = sum_f W2s[f, d] * actT[f, c]   (partial over f)
Partial outputs are written bf16 to halve write traffic (error impact
measured negligible: the 8 partials' rounding errors RSS to ~one bf16
rounding of the full sum).
"""

import math
import sys

sys.path.insert(0, "/opt/trn_rl_repo")

import ml_dtypes
import numpy as np

B, T, D, F, E = 2, 2048, 1024, 4096, 8
N = B * T
P = 128
KD = D // P  # 8
FS = F // 8  # 512 per-core f-slice
KFS = FS // P  # 4

bf16 = ml_dtypes.bfloat16

_nc_cache: dict[tuple, object] = {}
LAST_RESULTS = None
TRACE = False


def _expert_chunks(n: int) -> list[int]:
    """Near-equal chunks of <=512, all >=~256 so chunks never go
    LDWEIGHTS-bound (free dim >= 128 keeps the weight load hidden)."""
    if n == 0:
        return []
    k = math.ceil(n / 512)
    base = n // k
    rem = n - base * k
    return [base + (1 if i < rem else 0) for i in range(k)]


def _build(chunk_plan: tuple[tuple[int, ...], ...]):
    """chunk_plan[e] = tuple of chunk sizes for expert e's routed tokens."""
    import concourse.mybir as mybir
    import concourse.tile as tile
    from concourse import bacc

    dt = mybir.dt

    nc = bacc.Bacc(None, target_bir_lowering=False)

    flat = [(e, cn) for e in range(E) for cn in chunk_plan[e]]
    ntot = sum(cn for _, cn in flat)

    tokts = [
        nc.dram_tensor(f"tokt{i}", [P, KD, cn], dt.bfloat16, kind="ExternalInput")
        for i, (_, cn) in enumerate(flat)
    ]
    # f-major, expert-major weight slices: [P, E, KFS, KD, P]
    w1 = nc.dram_tensor("w1", [P, E, KFS, KD, P], dt.bfloat16, kind="ExternalInput")
    w2 = nc.dram_tensor("w2", [P, E, KFS, KD, P], dt.bfloat16, kind="ExternalInput")
    # [dm, p, c] so a whole chunk ([P, KD, cn] staged in SBUF) flushes as
    # ONE DMA: per-DMA issue cost and semaphore-reuse round-trips on the
    # sync queue were the dominant stall source with per-dm-tile DMAs
    out = nc.dram_tensor("out", [KD, P, ntot], dt.bfloat16, kind="ExternalOutput")

    with tile.TileContext(nc) as tc:
        with (
            tc.tile_pool(name="const", bufs=1) as cpool,
            tc.tile_pool(name="tok", bufs=4) as tpool,
            tc.tile_pool(name="act", bufs=2) as apool,
            tc.tile_pool(name="warm", bufs=1) as wpool,
            tc.tile_pool(name="ps1", bufs=3, space="PSUM") as ps1pool,
            tc.tile_pool(name="ps2", bufs=4, space="PSUM") as ps2pool,
            tc.tile_pool(name="psw", bufs=1, space="PSUM") as pswpool,
            tc.tile_pool(name="ob", bufs=3) as opool,
        ):
            w1_sb = cpool.tile([P, E, KFS, KD, P], dt.bfloat16, tag="w1")
            w2_sb = cpool.tile([P, E, KFS, KD, P], dt.bfloat16, tag="w2")

            # ---- PE warmup out of low p-state while first DMAs land ----
            warm_sb = wpool.tile([P, 512], dt.bfloat16, tag="warm")
            nc.gpsimd.memset(warm_sb[:], 0.0)
            ps_w = pswpool.tile([P, 512], dt.float32, tag="psw")
            NWARM = 7
            for wi in range(NWARM):
                nc.tensor.matmul(
                    ps_w[:],
                    warm_sb[:, 0:P],
                    warm_sb[:],
                    start=(wi == 0),
                    stop=(wi == NWARM - 1),
                )

            # ---- interleaved DMA issue + compute emission. Weight demand
            # is spread 8x thinner than expert-parallel: each expert
            # segment only needs its 2MB (W1s+W2s) before its chunks run.
            # W1 on the sync DGE queue, W2 on the scalar DGE queue, token
            # chunks double-buffered through a 4-deep ring.
            c0 = 0
            nflat = len(flat)
            prev_e = -1

            # Token DMA issues must be emitted BEFORE the out-DMA issues of
            # the chunk ~2 ahead of their use: the sync queue is in-order,
            # and out-DMA issues wait on their copy semaphores, so a token
            # issue placed after them only fires when that chunk's compute
            # is already done. Pre-issue chunks 0-2, then at chunk i
            # (right after stage 1, before chunk i's out-DMAs) issue
            # chunk i+3 into the 4-deep ring.
            pending: dict[int, object] = {}
            for j in range(min(3, nflat)):
                t_sb = tpool.tile([P, KD, 512], dt.bfloat16, tag="tok")
                cnj = flat[j][1]
                if j == 0:
                    # gate the first matmul on ~350KB: tok0's dk=0 slice +
                    # the first W1 f-block
                    nc.sync.dma_start(t_sb[:, 0:1, :cnj], tokts[0][:, 0:1])
                    nc.sync.dma_start(w1_sb[:, 0, 0:1], w1[:, 0, 0:1])
                    nc.sync.dma_start(t_sb[:, 1:KD, :cnj], tokts[0][:, 1:KD])
                    nc.sync.dma_start(w1_sb[:, 0, 1:KFS], w1[:, 0, 1:KFS])
                    nc.sync.dma_start(w2_sb[:, 0], w2[:, 0])
                else:
                    nc.sync.dma_start(t_sb[:, :, :cnj], tokts[j][:])
                pending[j] = t_sb

            for i, (e, cn) in enumerate(flat):
                tok_sb = pending.pop(i)
                act_sb = apool.tile([P, KFS, 512], dt.bfloat16, tag="act")
                # ---- stage 1 ----
                for fj in range(KFS):
                    ps1 = ps1pool.tile([P, cn], dt.float32, tag="ps1")
                    for dk in range(KD):
                        nc.tensor.matmul(
                            ps1[:],
                            w1_sb[:, e, fj, dk, :],
                            tok_sb[:, dk, :cn],
                            start=(dk == 0),
                            stop=(dk == KD - 1),
                        )
                    nc.scalar.activation(
                        act_sb[:, fj, :cn],
                        ps1[:],
                        mybir.ActivationFunctionType.Silu,
                    )
                # issue chunk i+3's token DMA and (at expert boundaries)
                # the next expert's weight prefetch here — after stage 1,
                # before this chunk's out-DMA issues enter the sync queue
                j = i + 3
                if j < nflat:
                    t_sb = tpool.tile([P, KD, 512], dt.bfloat16, tag="tok")
                    nc.sync.dma_start(t_sb[:, :, : flat[j][1]], tokts[j][:])
                    pending[j] = t_sb
                if e != prev_e and e + 1 < E:
                    nc.sync.dma_start(w1_sb[:, e + 1], w1[:, e + 1])
                    nc.sync.dma_start(w2_sb[:, e + 1], w2[:, e + 1])
                prev_e = e
                # ---- stage 2 (partial over this core's f-slice) ----
                ob = opool.tile([P, KD, 512], dt.bfloat16, tag="ob")
                for dm in range(KD):
                    ps2 = ps2pool.tile([P, cn], dt.float32, tag="ps2")
                    for fk in range(KFS):
                        nc.tensor.matmul(
                            ps2[:],
                            w2_sb[:, e, fk, dm, :],
                            act_sb[:, fk, :cn],
                            start=(fk == 0),
                            stop=(fk == KFS - 1),
                        )
                    # alternate PSUM->SBUF copies between vector and scalar
                    # (gpsimd cannot read PSUM): stage-2 groups are only 4
                    # matmuls (~0.7us) so a single copy engine would run
                    # ~85% busy. Copies land in the chunk staging tile.
                    if dm % 2 == 0:
                        nc.vector.tensor_copy(ob[:, dm, :cn], ps2[:])
                    else:
                        nc.scalar.activation(
                            ob[:, dm, :cn], ps2[:],
                            mybir.ActivationFunctionType.Copy,
                        )
                    if i == nflat - 1:
                        # last chunk: drain each dm tile immediately so the
                        # only post-final-matmul work is one small DMA
                        nc.sync.dma_start(
                            out[dm, :, c0 : c0 + cn], ob[:, dm, :cn]
                        )
                if i < nflat - 1:
                    # one flush DMA per chunk (per-dm-tile DMAs put ~9
                    # issues/chunk plus semaphore-reuse round-trips on the
                    # sync queue and paced the whole pipeline)
                    nc.sync.dma_start(
                        out[:, :, c0 : c0 + cn].rearrange("k p c -> p k c"),
                        ob[:, :, :cn],
                    )
                c0 += cn

    nc.compile()
    return nc


def _get_nc(chunk_plan):
    if chunk_plan not in _nc_cache:
        _nc_cache[chunk_plan] = _build(chunk_plan)
    return _nc_cache[chunk_plan]


def kernel(**inputs) -> np.ndarray:
    global LAST_RESULTS
    x = np.asarray(inputs["x"], dtype=np.float32)
    Wg = np.asarray(inputs["Wg"], dtype=np.float32)
    W1 = np.asarray(inputs["W1"], dtype=np.float32)
    W2 = np.asarray(inputs["W2"], dtype=np.float32)

    h = np.ascontiguousarray(x.reshape(N, D))

    # ---- host gate: top-2 + softmax ----
    logits = h @ Wg.T
    idx2 = np.argpartition(-logits, 1, axis=1)[:, :2]
    lsel = np.take_along_axis(logits, idx2, axis=1)
    first = lsel[:, 0] >= lsel[:, 1]
    i0 = np.where(first, idx2[:, 0], idx2[:, 1])
    i1 = np.where(first, idx2[:, 1], idx2[:, 0])
    l0 = np.where(first, lsel[:, 0], lsel[:, 1])
    l1 = np.where(first, lsel[:, 1], lsel[:, 0])
    e1 = np.exp((l1 - l0).astype(np.float32))
    w0 = (1.0 / (1.0 + e1)).astype(np.float32)
    w1g = (e1 / (1.0 + e1)).astype(np.float32)

    token_ids = np.concatenate([np.arange(N), np.arange(N)])
    expert_ids = np.concatenate([i0, i1])
    gate_w = np.concatenate([w0, w1g])

    # expert-sorted pair order
    order = np.argsort(expert_ids, kind="stable")
    sorted_tokens = token_ids[order]
    sorted_gw = gate_w[order]
    counts = np.bincount(expert_ids, minlength=E)

    chunk_plan = tuple(tuple(_expert_chunks(int(n))) for n in counts)
    flat = [cn for e in range(E) for cn in chunk_plan[e]]
    ntot = sum(flat)
    assert ntot == 2 * N

    hb = h.astype(bf16)
    # tokT for the full sorted pair stream: [P, KD, ntot]
    tokT = np.ascontiguousarray(
        hb[sorted_tokens].T.reshape(KD, P, ntot).transpose(1, 0, 2)
    )

    # weights, f-major expert-major: [P, E, KF, KD, P] then slice per core
    W1b = W1.astype(bf16)  # [E, D, F]
    W2b = W2.astype(bf16)  # [E, F, D]
    W1t = np.ascontiguousarray(
        W1b.reshape(E, KD, P, F // P, P).transpose(2, 0, 3, 1, 4)
    )  # [P, E, KF, KD, P]
    W2t = np.ascontiguousarray(
        W2b.reshape(E, F // P, P, KD, P).transpose(2, 0, 1, 3, 4)
    )  # [P, E, KF, KD, P]

    tok_arrays = {}
    c0 = 0
    for i, cn in enumerate(flat):
        tok_arrays[f"tokt{i}"] = np.ascontiguousarray(tokT[:, :, c0 : c0 + cn])
        c0 += cn

    in_maps = []
    for c in range(8):
        m = dict(tok_arrays)
        m["w1"] = np.ascontiguousarray(W1t[:, :, c * KFS : (c + 1) * KFS])
        m["w2"] = np.ascontiguousarray(W2t[:, :, c * KFS : (c + 1) * KFS])
        in_maps.append(m)

    nc = _get_nc(chunk_plan)
    from concourse.bass_utils import run_bass_kernel_spmd

    LAST_RESULTS = run_bass_kernel_spmd(
        nc, in_maps, core_ids=list(range(8)), trace=TRACE
    )

    # sum the 8 partials, then scatter-add with gate weights. Each token
    # appears exactly twice in the pair stream, so a stable sort by token
    # id groups its two rows adjacently for a reshape-sum.
    tot = np.zeros((D, ntot), dtype=np.float32)
    for c in range(8):
        o = np.asarray(LAST_RESULTS.results[c]["out"])  # [KD, P, ntot]
        tot += o.reshape(D, ntot).astype(np.float32)
    weighted = sorted_gw[:, None] * tot.T  # [ntot, D]
    inv = np.argsort(sorted_tokens, kind="stable")
    y = weighted[inv].reshape(N, 2, D).sum(axis=1)
    return y.reshape(B, T, D)
